# revision 30
# baseline (speedup 1.0000x reference)
"""GraphSAGE 2-layer (mean aggr) on 8 Trainium2 NeuronCores.

The workload is tiny for the hardware; the axon tunnel's host<->device
transfer (~30-40 MB/s, large per-array fixed cost) dominates wall time.
Everything is organized to minimize transferred bytes and array count:

  - 1D node partitioning: 8 cores each own 12544 (padded from 12500)
    destination rows; edges are owned by their dst core.
  - ALL per-core inputs ship as ONE packed int16 blob (~2.4 MB/core):
    fp16 x shard (p-major [128, NT, D]) | gather idx | scatter idx |
    1/deg | weights | biases. x is AllGathered ON DEVICE into the full
    f32 gather table (never replicated over the tunnel).
  - Aggregation: dma_gather of source rows (per-edge, 256B descriptors)
    followed by dma_scatter_add into a DRAM accumulator.
    dma_scatter_add races on colliding indices within one instruction, so
    edges are partitioned into "rounds" with at most one edge per dst row;
    rounds rotate over NA accumulator buffers (Tile's WAW dependency chain
    serializes same-buffer rounds, which is exactly what correctness needs).
  - SAGE transform on-chip per 128-row tile: cat = [agg*inv_deg | x_tile],
    catT = PE-transpose(cat), out = catT.T @ [W_l; W_r] + b — one matmul,
    no transposed-x input stream needed.
  - AllGather of layer-1 activations between the two convs.
  - Output returns as fp16 (rel err ~3e-4 vs the f32 reference overall).
"""

import numpy as np

F16 = np.float16

N = 100000
E = 1200000
D = 64
P = 8
NL = 12500          # real rows per core
NLP = 12544         # padded rows per core (= 98 * 128)
NT = NLP // 128     # 98 tiles of 128 rows
NG = NLP * P        # 100352 padded global rows
Q = 4               # gather table quadrants (int16 index limit)
QR = NG // Q        # 25088 rows per quadrant (= 2 cores' blocks)
DUMMY_DST = NLP - 1         # local junk row for scatter padding
PAD_SRC_LOCAL = (NL % 128) * NT + NL // 128   # p-major index of a zero row
NA = 4              # accumulator buffers (parallel scatter chains)
CHUNK = 128         # slot padding granule (gather out-slice granularity)
ST_SUPER = 7        # phase-B supertile = 7 x 128 rows (98 = 14*7)

_PROG_CACHE = {}
TRACE = False       # set True from test harness to collect a profile
_LAST_RESULT = [None]


def _build_host_data(x, edge_index, W1_l, b1, W1_r, W2_l, b2, W2_r):
    src = np.asarray(edge_index[0], dtype=np.int64)
    dst = np.asarray(edge_index[1], dtype=np.int64)
    x = np.asarray(x, dtype=np.float32)
    ne = dst.size

    owner = dst // NL
    dloc = dst - owner * NL
    cs = src // NL
    rloc = src - cs * NL
    gp = cs * NLP + (rloc % 128) * NT + rloc // 128   # p-major padded row

    # rank of each edge within its destination node (stable by edge order)
    deg_g = np.bincount(dst, minlength=N)
    o1 = np.argsort(dst.astype(np.int32), kind="quicksort")
    starts = np.zeros(N + 1, np.int64)
    np.cumsum(deg_g, out=starts[1:])
    rank = np.empty(ne, np.int64)
    rank[o1] = np.arange(ne) - starts[dst[o1]]
    R = max(int(deg_g.max()), NA)       # at least one round per acc buffer

    rnd = (rank + dloc) % R
    quad = gp // QR
    key = ((owner * R + rnd) * Q + quad) * (NG + 1) + gp
    if ((P * R) * Q) * (NG + 1) < 2**31:
        key = key.astype(np.int32)      # int32 sorts ~25% faster
    o2 = np.argsort(key, kind="quicksort")
    ow2, rnd2, quad2, gp2, d2 = owner[o2], rnd[o2], quad[o2], gp[o2], dloc[o2]

    cnt = np.bincount((ow2 * R + rnd2) * Q + quad2,
                      minlength=P * R * Q).reshape(P, R, Q)
    prq = ((cnt.max(axis=0) + CHUNK - 1) // CHUNK) * CHUNK      # [R, Q]
    srq = prq.sum(axis=1)                                       # [R]
    ST = int(srq.sum())
    flat = prq.reshape(-1)
    offs_q = (np.cumsum(flat) - flat).reshape(R, Q)             # slot offset of (r,q)
    roff = np.zeros(R + 1, np.int64)
    np.cumsum(srq, out=roff[1:])

    structure = (R, tuple(map(tuple, prq.tolist())))

    # slot of each edge: (r,q) segment base + rank within its (core,r,q) group
    # (groups are contiguous in the key-sorted stream)
    grpkey = (ow2 * R + rnd2) * Q + quad2
    changes = np.empty(ne, np.bool_)
    changes[0] = True
    changes[1:] = grpkey[1:] != grpkey[:-1]
    grp_start = np.maximum.accumulate(np.where(changes, np.arange(ne), 0))
    within = np.arange(ne) - grp_start
    slot = offs_q[rnd2, quad2] + within

    gstream = np.full((P, ST), PAD_SRC_LOCAL, np.int16)
    sstream = np.full((P, ST), DUMMY_DST, np.int16)
    fidx = ow2 * ST + slot
    gstream.reshape(-1)[fidx] = (gp2 % QR).astype(np.int16)
    sstream.reshape(-1)[fidx] = ((d2 % 128) * NT + d2 // 128).astype(np.int16)
    gw = np.ascontiguousarray(gstream.reshape(P, -1, 16).transpose(0, 2, 1))
    sw = np.ascontiguousarray(sstream.reshape(P, -1, 16).transpose(0, 2, 1))

    degp = np.zeros((P, NLP), np.int64)
    degp[:, :NL] = deg_g.reshape(P, NL)
    invc = np.ascontiguousarray(
        (1.0 / np.maximum(degp, 1)).astype(np.float32)
        .reshape(P, NT, 128).transpose(0, 2, 1))                # [P, 128, NT]

    xp = np.zeros((P, NLP, D), np.float32)
    xp[:, :NL] = x.reshape(P, NL, D)
    x_pm = np.ascontiguousarray(
        xp.reshape(P, NT, 128, D).transpose(0, 2, 1, 3)).astype(F16)

    b1r = np.broadcast_to(b1.astype(np.float32), (128, D))
    b2r = np.broadcast_to(b2.astype(np.float32), (128, D))
    ball = np.ascontiguousarray(np.concatenate([b1r, b2r], axis=1))
    wcat = np.ascontiguousarray(
        np.concatenate(
            [np.concatenate([W1_l, W1_r], axis=0),
             np.concatenate([W2_l, W2_r], axis=0)], axis=1), np.float32)

    # pack all per-core inputs into ONE int16 blob: the axon tunnel has a
    # large fixed cost per transferred array, so one array beats six.
    # segment order (i16 elems): x16 | gidx | sidx | invc | wcat | ball
    # (all f32 segments land at even i16 offsets for bitcast alignment)
    segs = [x_pm.view(np.int16).reshape(P, -1),
            gw.reshape(P, -1),
            sw.reshape(P, -1),
            invc.view(np.int16).reshape(P, -1),
            np.broadcast_to(wcat.view(np.int16).reshape(1, -1), (P, 32768)),
            np.broadcast_to(ball.view(np.int16).reshape(1, -1), (P, 32768))]
    blob = np.ascontiguousarray(np.concatenate(segs, axis=1))

    in_maps = [{"blob": blob[c]} for c in range(P)]
    counts = (cnt, prq, offs_q, roff)
    return structure, in_maps, counts, ST


def _build_program(structure, ST, counts):
    import os
    from concourse import bacc, mybir, tile
    from concourse.masks import make_identity

    max_rounds = int(os.environ.get("GNN_MAX_ROUNDS", "9999"))
    skip_cc = os.environ.get("GNN_SKIP_CC", "") == "1"
    skip_b = os.environ.get("GNN_SKIP_PHASEB", "") == "1"

    f32 = mybir.dt.float32
    f16 = mybir.dt.float16
    i16 = mybir.dt.int16
    R, prq_t = structure
    prq = np.array(prq_t, np.int64)
    cnt, _prq, offs_q, roff = counts

    nc = bacc.Bacc("TRN2", target_bir_lowering=False, debug=False, num_devices=P)
    # one packed input blob (i16 elems): x16 | gidx | sidx | invc | wcat | ball
    NX = 128 * NT * D
    NI = 128 * NT * 2
    NW = 32768
    o_x, o_g, o_s, o_i, o_w, o_b = np.cumsum(
        [0, NX, ST, ST, NI, NW])[:6].tolist()
    BLOB = o_b + NW
    t_blob = nc.dram_tensor("blob", [BLOB], i16, kind="ExternalInput")
    t_out = nc.dram_tensor("out", [128, NT, D], f16, kind="ExternalOutput")

    accs = [[nc.dram_tensor(f"acc{li}_{a}", [128, NT, D], f32) for a in range(NA)]
            for li in range(2)]
    x_shard = nc.dram_tensor("x_shard", [128, NT, D], f32)
    x_full = nc.dram_tensor("x_full", [NG, D], f32)
    h_shard = nc.dram_tensor("h_shard", [128, NT, D], f32)
    h_full = nc.dram_tensor("h_full", [NG, D], f32)

    NZ = 14                    # zero-fill tile width (NT = 98 = 7*14)
    with tile.TileContext(nc) as tc:
        with tc.tile_pool(name="persist", bufs=1) as pp, \
             tc.tile_pool(name="rounds", bufs=3) as rp, \
             tc.tile_pool(name="phaseb", bufs=2) as bp, \
             tc.tile_pool(name="psum_t", bufs=2, space="PSUM") as ptp, \
             tc.tile_pool(name="psum_o", bufs=2, space="PSUM") as pop:

            ZC = 49                 # acc zero-fill chunk (NT = 98 = 2*49)
            gidx_sb = pp.tile([128, ST // 16], i16)
            sidx_sb = pp.tile([128, ST // 16], i16)
            invc_sb = pp.tile([128, NT], f32)
            zero_sb = pp.tile([128, ZC, D], f32)
            wcat_sb = pp.tile([128, 2 * D], f32)
            ball_sb = pp.tile([128, 2 * D], f32)
            ident = pp.tile([128, 128], f32)
            x16_sb = pp.tile([128, NT, D], f16)
            h16_sb = pp.tile([128, NT, D], f16)

            for k in range(8):
                nc.sync.dma_start(out=gidx_sb[16 * k:16 * (k + 1), :],
                                  in_=t_blob[o_g:o_g + ST])
                nc.sync.dma_start(out=sidx_sb[16 * k:16 * (k + 1), :],
                                  in_=t_blob[o_s:o_s + ST])
            nc.sync.dma_start(out=invc_sb[:],
                              in_=t_blob[o_i:o_i + NI].bitcast(f32))
            nc.sync.dma_start(out=wcat_sb[:],
                              in_=t_blob[o_w:o_w + NW].bitcast(f32))
            nc.sync.dma_start(out=ball_sb[:],
                              in_=t_blob[o_b:o_b + NW].bitcast(f32))
            nc.sync.dma_start(out=x16_sb[:],
                              in_=t_blob[o_x:o_x + NX].bitcast(f16))
            make_identity(nc, ident[:])
            nc.vector.memset(zero_sb[:], 0.0)

            # upcast x shard to f32 and stage to DRAM for the AllGather
            for z in range(NT // NZ):
                t0 = z * NZ
                xc = bp.tile([128, NZ, D], f32, tag="xcvt", name=f"xc{z}")
                nc.vector.tensor_copy(out=xc[:], in_=x16_sb[:, t0:t0 + NZ, :])
                nc.sync.dma_start(out=x_shard[:, t0:t0 + NZ, :], in_=xc[:])
            if not skip_cc:
                nc.gpsimd.collective_compute(
                    "AllGather",
                    mybir.AluOpType.bypass,
                    replica_groups=[list(range(P))],
                    ins=[x_shard.ap().opt()],
                    outs=[x_full.ap().opt()],
                )

            for li in range(2):
                table = x_full if li == 0 else h_full
                for a in range(NA):
                    for z in range(NT // ZC):
                        nc.sync.dma_start(
                            out=accs[li][a][:, z * ZC:(z + 1) * ZC, :],
                            in_=zero_sb[:])

                MAXTOK = 1024       # per-instruction token cap (>=2048 hangs
                                    # the device: SWDGE ring capacity)
                for r in range(min(R, max_rounds)):
                    s_r = int(prq[r].sum())
                    if s_r == 0:
                        continue
                    rt = rp.tile([128, s_r // 128, D], f32, tag="roundtile",
                                 name=f"rt{li}_{r}")
                    c0 = 0
                    for q in range(Q):
                        s = int(prq[r, q])
                        off16 = int(offs_q[r, q]) // 16
                        for o in range(0, s, MAXTOK):
                            ss = min(MAXTOK, s - o)
                            nc.gpsimd.dma_gather(
                                rt[:, c0 + o // 128: c0 + (o + ss) // 128, :],
                                table[q * QR:(q + 1) * QR, :],
                                gidx_sb[:, off16 + o // 16: off16 + (o + ss) // 16],
                                ss, ss, D)
                        c0 += s // 128
                    soff16 = int(roff[r]) // 16
                    for o in range(0, s_r, MAXTOK):
                        ss = min(MAXTOK, s_r - o)
                        nc.gpsimd.dma_scatter_add(
                            accs[li][r % NA][:].flatten_outer_dims(),
                            rt[:, o // 128:(o + ss) // 128, :],
                            sidx_sb[:, soff16 + o // 16: soff16 + (o + ss) // 16],
                            ss, ss, D)

                wl = wcat_sb[:, li * D:(li + 1) * D]
                bb = ball_sb[:, li * D:(li + 1) * D]
                for st in range(0 if skip_b else NT // ST_SUPER):
                    t0 = st * ST_SUPER
                    ac = []
                    for a in range(NA):
                        at = bp.tile([128, ST_SUPER, D], f32, tag=f"acc_ld{a}",
                                     name=f"at{li}_{st}_{a}")
                        nc.sync.dma_start(out=at[:],
                                          in_=accs[li][a][:, t0:t0 + ST_SUPER, :])
                        ac.append(at)
                    agg = bp.tile([128, ST_SUPER, D], f32, tag="agg",
                                  name=f"agg{li}_{st}")
                    nc.vector.tensor_tensor(out=agg[:], in0=ac[0][:], in1=ac[1][:],
                                            op=mybir.AluOpType.add)
                    for a in range(2, NA):
                        nc.vector.tensor_tensor(out=agg[:], in0=agg[:], in1=ac[a][:],
                                                op=mybir.AluOpType.add)
                    # cat tile: [agg * inv_deg | x or h]
                    cat = bp.tile([128, ST_SUPER, 2 * D], f32, tag="cat",
                                  name=f"cat{li}_{st}")
                    nc.vector.tensor_tensor(
                        out=cat[:, :, 0:D], in0=agg[:],
                        in1=invc_sb[:, t0:t0 + ST_SUPER].unsqueeze(-1).to_broadcast(
                            [128, ST_SUPER, D]),
                        op=mybir.AluOpType.mult)
                    side = x16_sb if li == 0 else h16_sb
                    nc.scalar.activation(
                        out=cat[:, :, D:2 * D], in_=side[:, t0:t0 + ST_SUPER, :],
                        func=mybir.ActivationFunctionType.Identity)
                    # transpose the 7 cat tiles into two psum banks, copy to
                    # SBUF in two strokes, one matmul per tile into a shared
                    # psum bank, then a single biased epilogue.
                    ptA = ptp.tile([128, 4 * 128], f32, tag="tp4",
                                   name=f"ptA{li}_{st}")
                    ptB = ptp.tile([128, 3 * 128], f32, tag="tp3",
                                   name=f"ptB{li}_{st}")
                    for j in range(ST_SUPER):
                        dst = ptA if j < 4 else ptB
                        k = j if j < 4 else j - 4
                        nc.tensor.transpose(out=dst[:, k * 128:(k + 1) * 128],
                                            in_=cat[:, j, :], identity=ident[:])
                    catT = bp.tile([128, ST_SUPER * 128], f32, tag="catT",
                                   name=f"catT{li}_{st}")
                    nc.vector.tensor_copy(out=catT[:, 0:512], in_=ptA[:])
                    nc.vector.tensor_copy(out=catT[:, 512:896], in_=ptB[:])
                    po = pop.tile([128, ST_SUPER, D], f32, tag="mo",
                                  name=f"po{li}_{st}")
                    for j in range(ST_SUPER):
                        nc.tensor.matmul(out=po[:, j, :],
                                         lhsT=catT[:, j * 128:(j + 1) * 128],
                                         rhs=wl, start=True, stop=True)
                    bbx = bb.unsqueeze(1).to_broadcast([128, ST_SUPER, D])
                    if li == 0:
                        res = bp.tile([128, ST_SUPER, D], f32, tag="res",
                                      name=f"res{li}_{st}")
                        nc.vector.tensor_tensor(out=res[:], in0=po[:], in1=bbx,
                                                op=mybir.AluOpType.add)
                        nc.scalar.activation(out=res[:], in_=res[:],
                                             func=mybir.ActivationFunctionType.Relu)
                        nc.vector.tensor_copy(out=h16_sb[:, t0:t0 + ST_SUPER, :],
                                              in_=res[:])
                        nc.sync.dma_start(out=h_shard[:, t0:t0 + ST_SUPER, :],
                                          in_=res[:])
                    else:
                        o16 = bp.tile([128, ST_SUPER, D], f16, tag="o16",
                                      name=f"o16_{st}")
                        nc.vector.tensor_tensor(out=o16[:], in0=po[:], in1=bbx,
                                                op=mybir.AluOpType.add)
                        nc.sync.dma_start(out=t_out[:, t0:t0 + ST_SUPER, :],
                                          in_=o16[:])

                if li == 0 and not skip_cc:
                    nc.gpsimd.collective_compute(
                        "AllGather",
                        mybir.AluOpType.bypass,
                        replica_groups=[list(range(P))],
                        ins=[h_shard.ap().opt()],
                        outs=[h_full.ap().opt()],
                    )

    nc.compile()
    return nc


_WARMED = [False]


def _warm_devices():
    """Tiny sharded device_put before the first real transfer — the first
    H2D in a process intermittently hits a multi-second (sometimes minutes)
    device-attach stall; a small put absorbs that."""
    if _WARMED[0]:
        return
    import jax
    from jax.sharding import Mesh, PartitionSpec, NamedSharding
    devs = jax.devices()[:P]
    mesh = Mesh(np.asarray(devs), ("core",))
    p = jax.device_put(np.ones((P * 8, 8), np.float32),
                       NamedSharding(mesh, PartitionSpec("core")))
    p.block_until_ready()
    _WARMED[0] = True


def _prewarm():
    try:
        import concourse.bass_utils  # noqa: F401  (heavy import chain)
        _warm_devices()
    except Exception:
        pass                         # retried inline from kernel()


import threading as _threading
_PREWARM_T = _threading.Thread(target=_prewarm, daemon=True)
_PREWARM_T.start()


def kernel(x, edge_index, W1_l, b1, W1_r, W2_l, b2, W2_r):
    import time as _time
    from concourse import bass_utils

    structure, in_maps, counts, ST = _build_host_data(
        x, edge_index, W1_l, b1, W1_r, W2_l, b2, W2_r)
    import os as _os
    key = (structure, ST, _os.environ.get("GNN_MAX_ROUNDS", ""),
           _os.environ.get("GNN_SKIP_CC", ""), _os.environ.get("GNN_SKIP_PHASEB", ""))
    if key not in _PROG_CACHE:
        _PROG_CACHE[key] = _build_program(structure, ST, counts)
    nc = _PROG_CACHE[key]

    _PREWARM_T.join()
    _warm_devices()                  # no-op unless the prewarm thread failed
    _t0 = _time.time()
    try:
        res = bass_utils.run_bass_kernel_spmd(
            nc, in_maps, list(range(P)), trace=TRACE)
    except ModuleNotFoundError:
        # axon NTFF profiling hook unavailable in this container
        res = bass_utils.run_bass_kernel_spmd(
            nc, in_maps, list(range(P)), trace=False)
    _LAST_RESULT[0] = res
    _LAST_RESULT.append(_time.time() - _t0)
    out = np.concatenate(
        [np.asarray(res.results[c]["out"]).astype(np.float32)
         .transpose(1, 0, 2).reshape(NLP, D)[:NL]
         for c in range(P)], axis=0)
    return out
